# revision 2
# baseline (speedup 1.0000x reference)
"""Trainium2 Bass kernel for StyleGAN2-style upsampling ConvLayer.

Reference computation (per image):
  y = conv_transpose2d(x, (w*WSCALE), stride=2)      # 512ch 64x64 -> 256ch 129x129
  y = upfirdn2d(y, fir([1,3,3,1]), pad=1, gain=4)    # 4x4 blur   -> 128x128
  y = clamp(lrelu(y + bias, 0.2) * sqrt(2), +-256)

Hybrid factorization (validated exact vs reference):
  - Fold the *horizontal* FIR into the weights:
      W_h[o,i,a,u] = (WSCALE*4/64) * sum_b w[o,i,a,b] * [1,3,3,1][u-b]   (3x6 taps)
  - PE computes the vertically-sparse intermediate zz (129 rows x 128 cols):
      zz[2P+s, 2Q+t] = sum_{ic,da,du} W_h[s+2da, t+2du] * x[ic, P-da, Q+1-du]
    as fp32r channel-contraction matmuls (24 or 12 per PSUM group of N=512).
  - DVE applies the vertical FIR [1,3,3,1] as three 2-tap box passes
    (binomial factorization), pure tensor_tensor adds.
  - ACT evacuates PSUM (column-interleaving parities) and applies the
    Prelu epilogue; DVE clamps; contiguous DMA out.

Sharding: data parallel, 2 images per core across 8 NeuronCores.
"""

import numpy as np

N_CORES = 8
IMG_PER_CORE = 2
IN_CH, OUT_CH, K, UP = 512, 256, 3, 2
H = W = 64
WSCALE = float(1.0 / np.sqrt(K * K * IN_CH))
ACT_GAIN = float(np.sqrt(2.0))
CLAMP = 256.0
ALPHA = 0.2
R = 8                  # parity rows per PE group -> matmul N = R*64 = 512
N_RB = H // R          # 8 full zz tiles (16 rows each) + 1 tail tile (2 rows)
N_ICC = IN_CH // 128   # 4 ic chunks
N_OCC = OUT_CH // 128  # 2 oc chunks

_CACHE = {}


def _prep_wh(weight: np.ndarray) -> np.ndarray:
    """wh[occhunk, ic, icchunk, a(3), u(6), oc] float32 with all FIR scales folded."""
    fir4 = np.array([1.0, 3.0, 3.0, 1.0], np.float64)
    w64 = weight.astype(np.float64) * (WSCALE * 4.0 / 64.0)
    W_h = np.zeros((OUT_CH, IN_CH, 3, 6), np.float64)
    for b in range(3):
        W_h[:, :, :, b:b + 4] += w64[:, :, :, b:b + 1] * fir4[None, None, None]
    arr = W_h.reshape(N_OCC, 128, N_ICC, 128, 3, 6)  # [oa, o, c, i, a, u]
    wh = np.ascontiguousarray(
        arr.transpose(0, 3, 2, 4, 5, 1).astype(np.float32))  # [oa, i, c, a, u, o]
    return wh


def _build_nc(n_img: int, n_occ: int, n_img_store: int | None = None):
    # n_img_store < n_img makes later images overwrite earlier output rows —
    # used only by timing harnesses to scale compute at fixed I/O shapes.
    if n_img_store is None:
        n_img_store = n_img
    import concourse.bacc as bacc
    import concourse.mybir as mybir
    import concourse.tile as tile

    f32 = mybir.dt.float32
    f32r = mybir.dt.float32r
    Prelu = mybir.ActivationFunctionType.Prelu
    Copy = mybir.ActivationFunctionType.Copy
    AluOp = mybir.AluOpType

    nc = bacc.Bacc()
    xp_ext = nc.declare_dram_parameter(
        "xp", [n_img, N_ICC, 128, H + 2, W + 2], f32, isOutput=False)
    wh_ext = nc.declare_dram_parameter(
        "wh", [N_OCC, 128, N_ICC, 3, 6, 128], f32, isOutput=False)
    bg_ext = nc.declare_dram_parameter("bg", [128, N_OCC], f32, isOutput=False)
    out_ext = nc.declare_dram_parameter(
        "out", [n_img_store, OUT_CH, 2 * H, 2 * W], f32, isOutput=True)

    with tile.TileContext(nc) as tc:
        with (
            tc.tile_pool(name="wpool", bufs=1) as wpool,
            tc.tile_pool(name="xpool", bufs=2) as xpool,
            tc.tile_pool(name="zpool", bufs=10) as zpool,
            tc.tile_pool(name="tpool", bufs=2) as tpool,
            tc.tile_pool(name="ypool", bufs=2) as ypool,
            tc.tile_pool(name="cpool", bufs=1) as cpool,
            tc.tile_pool(name="ppool", bufs=8, space="PSUM") as ppool,
        ):
            bt = cpool.tile([128, N_OCC], f32)
            nc.sync.dma_start(out=bt[:], in_=bg_ext[:])
            zrow = cpool.tile([128, 1, 2 * W], f32)  # zero boundary row
            nc.vector.memset(zrow[:], 0.0)

            for oa in range(n_occ):
                wt = wpool.tile([128, N_ICC * 3 * 6 * 128], f32r, tag="wt")
                nc.sync.dma_start(out=wt[:], in_=wh_ext[oa].bitcast(f32r))
                for img in range(n_img):
                    zz = [None] * (N_RB + 1)

                    def pe_tile(rb):
                        # x tiles: padded rows [8rb, 8rb+9) (or [64,66) for rb=8)
                        nrow = 9 if rb < N_RB else 2
                        r0 = rb * R
                        xts = []
                        for c in range(N_ICC):
                            xt = xpool.tile([128, nrow, W + 2], f32r, tag=f"x{c}")
                            nc.sync.dma_start(
                                out=xt[:],
                                in_=xp_ext[img, c, :, r0:r0 + nrow, :].bitcast(f32r))
                            xts.append(xt)
                        if rb < N_RB:
                            zt = zpool.tile([128, 2 * R, 2 * W], f32, tag="zz")
                        else:
                            zt = zpool.tile([128, 2, 2 * W], f32, tag="zz")
                            nc.vector.memset(zt[:], 0.0)
                        zz[rb] = zt
                        rows = R if rb < N_RB else 1
                        s_list = (0, 1) if rb < N_RB else (0,)
                        for s in s_list:
                            da_list = (0, 1) if s == 0 else (0,)
                            for t in range(2):
                                ps = ppool.tile([128, rows * W], f32, tag="ps")
                                nmm = len(da_list) * 3 * N_ICC
                                j = 0
                                for c in range(N_ICC):
                                    for da in da_list:
                                        a = s + 2 * da
                                        for du in range(3):
                                            idx = (c * 3 + a) * 6 + (t + 2 * du)
                                            rhs = xts[c][:, 1 - da:1 - da + rows,
                                                         2 - du:2 - du + W]
                                            nc.tensor.matmul(
                                                ps[:],
                                                wt[:, idx * 128:(idx + 1) * 128],
                                                rhs,
                                                start=(j == 0), stop=(j == nmm - 1))
                                            j += 1
                                if rb < N_RB:
                                    dst = zt[:].rearrange(
                                        "p (r s) (q t) -> p s t r q",
                                        s=2, t=2)[:, s, t]
                                    src = ps[:].rearrange("p (r q) -> p r q", r=rows)
                                else:
                                    dst = zt[:].rearrange(
                                        "p r (q t) -> p t r q", t=2)[:, t, 0:1]
                                    src = ps[:].rearrange("p (r q) -> p r q", r=1)
                                nc.scalar.activation(dst, src, Copy)

                    def dve_block(ob):
                        # out rows [16ob, 16ob+16); needs zz rows [16ob-1, 16ob+18)
                        t1 = tpool.tile([128, 18, 2 * W], f32, tag="t1")
                        prev = zrow[:] if ob == 0 else zz[ob - 1][:, 15:16]
                        nc.vector.tensor_tensor(
                            t1[:, 0:1], zz[ob][:, 0:1], prev, AluOp.add)
                        nc.vector.tensor_tensor(
                            t1[:, 1:16], zz[ob][:, 1:16], zz[ob][:, 0:15], AluOp.add)
                        nc.vector.tensor_tensor(
                            t1[:, 16:17], zz[ob + 1][:, 0:1], zz[ob][:, 15:16], AluOp.add)
                        nc.vector.tensor_tensor(
                            t1[:, 17:18], zz[ob + 1][:, 1:2], zz[ob + 1][:, 0:1], AluOp.add)
                        t2 = tpool.tile([128, 17, 2 * W], f32, tag="t2")
                        nc.gpsimd.tensor_tensor(
                            t2[:], t1[:, 1:18], t1[:, 0:17], AluOp.add)
                        yt = ypool.tile([128, 16, 2 * W], f32, tag="yt")
                        nc.vector.tensor_tensor(
                            yt[:], t2[:, 0:16], t2[:, 1:17], AluOp.add)
                        nc.scalar.activation(yt[:], yt[:], Prelu,
                                             bias=bt[:, oa:oa + 1],
                                             scale=ACT_GAIN, alpha=ALPHA)
                        nc.gpsimd.tensor_scalar(
                            yt[:], yt[:], CLAMP, -CLAMP, AluOp.min, AluOp.max)
                        nc.sync.dma_start(
                            out=out_ext[img % n_img_store, oa * 128:(oa + 1) * 128,
                                        16 * ob:16 * ob + 16, :],
                            in_=yt[:])

                    for rb in range(N_RB + 1):
                        pe_tile(rb)
                        if rb >= 1:
                            dve_block(rb - 1)
    nc.compile()
    return nc


def build_null_like_nc():
    """Same DRAM I/O signature as the real kernel, trivial body (for bench null)."""
    import concourse.bacc as bacc
    import concourse.mybir as mybir
    import concourse.tile as tile
    f32 = mybir.dt.float32
    nc = bacc.Bacc()
    nc.declare_dram_parameter(
        "xp", [IMG_PER_CORE, N_ICC, 128, H + 2, W + 2], f32, isOutput=False)
    nc.declare_dram_parameter(
        "wh", [N_OCC, 128, N_ICC, 3, 6, 128], f32, isOutput=False)
    bg_ext = nc.declare_dram_parameter("bg", [128, N_OCC], f32, isOutput=False)
    out_ext = nc.declare_dram_parameter(
        "out", [IMG_PER_CORE, OUT_CH, 2 * H, 2 * W], f32, isOutput=True)
    with tile.TileContext(nc) as tc:
        with tc.tile_pool(name="p", bufs=1) as p:
            t = p.tile([128, N_OCC], f32)
            nc.sync.dma_start(out=t[:], in_=bg_ext[:])
            nc.sync.dma_start(out=out_ext[0, 0:128, 0, 0:N_OCC], in_=t[:])
    nc.compile()
    return nc


def _get_nc(n_img: int, n_occ: int):
    key = (n_img, n_occ)
    if key not in _CACHE:
        _CACHE[key] = _build_nc(n_img, n_occ)
    return _CACHE[key]


def kernel(x: np.ndarray, weight: np.ndarray, bias: np.ndarray) -> np.ndarray:
    from concourse.bass_utils import run_bass_kernel_spmd

    x = np.asarray(x, np.float32)
    weight = np.asarray(weight, np.float32)
    bias = np.asarray(bias, np.float32)

    wh = _prep_wh(weight)
    bg = np.ascontiguousarray(
        (bias.astype(np.float64) * ACT_GAIN).astype(np.float32)
        .reshape(N_OCC, 128).T)

    n_total = x.shape[0]
    xq = x.reshape(n_total, N_ICC, 128, H, W)
    xpad = np.zeros((n_total, N_ICC, 128, H + 2, W + 2), np.float32)
    xpad[:, :, :, 1:H + 1, 1:W + 1] = xq

    nc = _get_nc(IMG_PER_CORE, N_OCC)
    in_maps = []
    for c in range(N_CORES):
        sl = np.ascontiguousarray(xpad[c * IMG_PER_CORE:(c + 1) * IMG_PER_CORE])
        in_maps.append({"xp": sl, "wh": wh, "bg": bg})
    res = run_bass_kernel_spmd(nc, in_maps, list(range(N_CORES)))
    out = np.concatenate([res.results[c]["out"] for c in range(N_CORES)], axis=0)
    return out

